# revision 27
# baseline (speedup 1.0000x reference)
"""GQA attention kernel for Trainium2, 8 NeuronCores.

Sharding: core = b*4 + g  (b = batch 0..1, g = kv-head group 0..3).
Each core handles one batch and one kv group (1 kv head + its 4 query
heads). wq/wo are split by head group (column/row), wk/wv by kv head.
The output projection partial sums (one per group) are reduced on the
host.

On-device layout is "transposed activation" space: activations are
[feature, seq] so every matmul contraction lands on SBUF partitions.
All matmul operands are bf16 (PSUM accumulation stays f32); end-to-end
relative error vs the f32 reference is ~5e-3 against a 2e-2 budget.

Per 512-column query chunk c:
  fused projection pass over the chunk's x tiles (loaded once, resident
  in SBUF):
    K^T  += wk_t^T x_t     [dh, s]
    Q0^T += wq_t^T x_t     (head 0; heads 1-3 are emitted later as PE
                            filler inside the previous head's attention)
    V    += x_t^T wv_t     [s, dh]  (computed pre-transposed so the AV
                            matmul needs no separate transpose)
  RoPE on [d, s] tiles: rope(Z) = Z*C + (Pswap @ Z)*Sg with the sign
  folded into the host-built Sg tile.
  attention per head h over 128-wide key tiles j (keys on partitions):
    S^T = K^T.T @ Q^T       [keys, queries]
    P   = exp(S^T / sqrt(dh))   (no max subtraction: scores are O(10))
    diagonal tiles: multiplicative causal mask on the 128-wide triangle
    d_s = P_slice^T @ 1     per 128-query slice -- a [128,1]-output
          matmul (~1 PE cycle vs 512 for a row-form ones-matmul)
    O^T += V.T @ P
    normalize: recip(d) -> PE transpose to a row -> gpsimd partition
    broadcast -> multiply on DVE.
  The output projection y^T = wo^T O^T of chunk c is emitted as PE
  filler inside chunk c+1's attention loops so exp latency never stalls
  the PE; chunk 3's runs at the end.

DMAs are batched (>=1KB contiguous runs) and issued in first-use order
on the SP queue; stores go through the Pool SWDGE queue.
"""

import sys

sys.path.insert(0, "/opt/trn_rl_repo")

from collections import deque
from contextlib import ExitStack

import numpy as np

import concourse.bass as bass
import concourse.tile as tile
from concourse import bacc, mybir
from concourse import bass_utils

F32 = mybir.dt.float32
BF16 = mybir.dt.bfloat16
MULT = mybir.AluOpType.mult
EXP = mybir.ActivationFunctionType.Exp

S = 2048          # sequence length
DM = 2048         # d_model
DH = 128          # head dim
HPC = 4           # query heads per core (= n_rep; one kv group per core)
N_CORES = 8
CH = 512          # query-chunk width
NCHUNK = S // CH  # 4
NT = DM // 128    # 16 contraction tiles of d_model
SCALE = 1.0 / float(np.sqrt(DH))

_CACHE = {}
_MARKS = []


def _build():
    import os
    cfg = lambda k, d: int(os.environ.get("GQA_" + k, d))
    _MARKS.clear()

    nc = bacc.Bacc("TRN2", target_bir_lowering=False, debug=False)

    xT = nc.dram_tensor("xT", [DM, S], BF16, kind="ExternalInput").ap()
    wq = nc.dram_tensor("wq", [DM, HPC * DH], BF16, kind="ExternalInput").ap()
    wkv = nc.dram_tensor("wkv", [DM, 2 * DH], BF16, kind="ExternalInput").ap()
    wo = nc.dram_tensor("wo", [HPC * DH, DM], BF16, kind="ExternalInput").ap()
    cssn = nc.dram_tensor("cssn", [128, 2 * S], BF16, kind="ExternalInput").ap()
    pswap = nc.dram_tensor("pswap", [DH, DH], BF16, kind="ExternalInput").ap()
    ident = nc.dram_tensor("ident", [128, 128], BF16, kind="ExternalInput").ap()
    masks = nc.dram_tensor("masks", [128, 4 * CH], BF16, kind="ExternalInput").ap()
    ones = nc.dram_tensor("ones", [128, 1], BF16, kind="ExternalInput").ap()
    yT = nc.dram_tensor("yT", [DM, S], BF16, kind="ExternalOutput").ap()

    with tile.TileContext(nc) as tc, ExitStack() as ctx:
        consts = ctx.enter_context(tc.tile_pool(name="consts", bufs=1))
        xpool = ctx.enter_context(tc.tile_pool(name="xpool", bufs=cfg("XP", 3)))
        qtp = ctx.enter_context(tc.tile_pool(name="qtp", bufs=cfg("QT", 2)))
        ppool = ctx.enter_context(tc.tile_pool(name="ppool", bufs=cfg("PP", 6)))
        rtmp = ctx.enter_context(tc.tile_pool(name="rtmp", bufs=cfg("RT", 2)))
        otp = ctx.enter_context(tc.tile_pool(name="otp", bufs=cfg("OT", 2)))
        ybigp = ctx.enter_context(tc.tile_pool(name="ybig", bufs=cfg("YB", 2)))
        misc = ctx.enter_context(tc.tile_pool(name="misc", bufs=cfg("MI", 2)))
        # PSUM: 8 banks total (sum over pool tags of bufs must stay <= 8).
        # k_acc shares the "ot" ring; v_ps shares the "st" ring.
        ps_q = ctx.enter_context(tc.tile_pool(name="ps_q", bufs=cfg("PQ", 1), space="PSUM"))
        ps_d = ctx.enter_context(tc.tile_pool(name="ps_d", bufs=cfg("PD", 1), space="PSUM"))
        ps_st = ctx.enter_context(tc.tile_pool(name="ps_st", bufs=cfg("PST", 2), space="PSUM"))
        ps_ot = ctx.enter_context(tc.tile_pool(name="ps_ot", bufs=cfg("POT", 2), space="PSUM"))
        ps_yt = ctx.enter_context(tc.tile_pool(name="ps_yt", bufs=cfg("PYT", 2), space="PSUM"))

        # ---------------- SBUF tiles ----------------
        wkv_sb = consts.tile([128, NT, 2 * DH], BF16, tag="wkv")
        wq_sb = consts.tile([128, NT, HPC * DH], BF16, tag="wq")
        wo_sb = consts.tile([128, HPC, DM], BF16, tag="wo")
        cssn_sb = consts.tile([128, 2 * S], BF16, tag="cssn")
        pswap_sb = consts.tile([DH, DH], BF16, tag="pswap")
        ident_sb = consts.tile([128, 128], BF16, tag="ident")
        masks_sb = consts.tile([128, 4, CH], BF16, tag="masks")
        ones_sb = consts.tile([128, 1], BF16, tag="ones")
        kt_sb = consts.tile([DH, S], BF16, tag="kt")       # roped K^T
        v_sb = consts.tile([128, S], BF16, tag="v")        # V in [s, dv], flat

        x_tiles = [xpool.tile([128, NT, CH], BF16, tag="x", name=f"x{c}")
                   for c in range(NCHUNK)]
        qt_tiles = [qtp.tile([128, HPC, CH], BF16, tag="qt", name=f"qt{c}")
                    for c in range(NCHUNK)]
        ot_tiles = [otp.tile([128, HPC, CH], BF16, tag="ot", name=f"ot{c}")
                    for c in range(NCHUNK)]
        ybig_tiles = [
            [ybigp.tile([128, 8, CH], BF16, tag="yb", name=f"yb{c}_{i}")
             for i in range(2)]
            for c in range(NCHUNK)
        ]

        # ---------------- input DMAs, first-use order (SP queue) ----------
        wkv_r = wkv.rearrange("(t p) n -> p t n", p=128)
        wq_r = wq.rearrange("(t p) n -> p t n", p=128)
        wo_r = wo.rearrange("(h p) n -> p h n", p=128)
        x_r = xT.rearrange("(t p) s -> p t s", p=128)
        yT_r = yT.rearrange("(d p) s -> p d s", p=128)

        def x_dma(c, g):
            col = c * CH
            nc.sync.dma_start(
                x_tiles[c][:, 4 * g:4 * g + 4, :],
                x_r[:, 4 * g:4 * g + 4, col:col + CH])

        nc.sync.dma_start(wkv_sb[:, 0:4, :], wkv_r[:, 0:4, :])
        x_dma(0, 0)
        nc.sync.dma_start(wq_sb[:, 0:4, :], wq_r[:, 0:4, :])
        nc.sync.dma_start(wkv_sb[:, 4:8, :], wkv_r[:, 4:8, :])
        x_dma(0, 1)
        nc.sync.dma_start(wq_sb[:, 4:8, :], wq_r[:, 4:8, :])
        nc.sync.dma_start(wkv_sb[:, 8:16, :], wkv_r[:, 8:16, :])
        x_dma(0, 2)
        nc.sync.dma_start(wq_sb[:, 8:12, :], wq_r[:, 8:12, :])
        x_dma(0, 3)
        nc.sync.dma_start(wq_sb[:, 12:16, :], wq_r[:, 12:16, :])
        nc.sync.dma_start(cssn_sb, cssn)
        nc.sync.dma_start(pswap_sb, pswap)
        nc.sync.dma_start(ones_sb, ones)
        nc.sync.dma_start(masks_sb, masks.rearrange("p (t n) -> p t n", t=4))
        nc.sync.dma_start(ident_sb, ident)
        for g in range(4):
            x_dma(1, g)
        nc.sync.dma_start(wo_sb[:, 0:2, :], wo_r[:, 0:2, :])
        nc.sync.dma_start(wo_sb[:, 2:4, :], wo_r[:, 2:4, :])
        for g in range(4):
            x_dma(2, g)
        for g in range(4):
            x_dma(3, g)

        # ---------------- helpers ----------------
        def rope_closures(acc_ps, c, out_ap, sw_ps, raw_on_act=True):
            """[raw_copy, sw_matmul, mults, final_add] closures.
            out = raw*C + (Pswap @ raw)*Sg for s-chunk c. Filler-head ropes
            copy raw on DVE so the hot exp stream on ACT never delays them."""
            col = c * CH
            raw = rtmp.tile([128, CH], BF16, tag="raw")
            ta = rtmp.tile([128, CH], F32, tag="ta")
            tb = rtmp.tile([128, CH], F32, tag="tb")
            cs_ap = cssn_sb[:, col:col + CH]
            sn_ap = cssn_sb[:, S + col:S + col + CH]
            raw_eng = (lambda: nc.scalar.copy(raw, acc_ps)) if raw_on_act \
                else (lambda: nc.vector.tensor_copy(raw, acc_ps))
            return [
                raw_eng,
                lambda: nc.tensor.matmul(sw_ps, pswap_sb, raw,
                                         start=True, stop=True,
                                         skip_group_check=True),
                lambda: (nc.gpsimd.tensor_tensor(ta, raw, cs_ap, MULT),
                         nc.vector.tensor_tensor(tb, sw_ps, sn_ap, MULT)),
                lambda: nc.vector.tensor_add(out_ap, ta, tb),
            ]

        def q_mm_closures(c, h, acc):
            units = []
            for t in range(NT):
                def qmm(t=t):
                    nc.tensor.matmul(
                        acc, wq_sb[:, t, h * DH:(h + 1) * DH],
                        x_tiles[c][:, t, :],
                        start=(t == 0), stop=(t == NT - 1),
                        skip_group_check=True)
                units.append(qmm)
            return units

        def op_store(c, dt, yt_ps, tail):
            half = dt // 8
            ybig = ybig_tiles[c][half]
            if dt % 2:
                nc.scalar.copy(ybig[:, dt % 8, :], yt_ps)
            else:
                nc.vector.tensor_copy(ybig[:, dt % 8, :], yt_ps)
            if tail:
                # finer stores on the idle SP queue shorten the drain
                if dt % 4 == 3:
                    q0 = (dt // 4) * 4
                    nc.sync.dma_start(
                        yT_r[:, q0:q0 + 4, c * CH:(c + 1) * CH],
                        ybig[:, q0 % 8:q0 % 8 + 4, :])
            elif dt % 8 == 7:
                d0 = half * 8
                nc.gpsimd.dma_start(
                    yT_r[:, d0:d0 + 8, c * CH:(c + 1) * CH], ybig)

        def op_mms(c, dt, yt_ps, hs):
            for h in hs:
                nc.tensor.matmul(
                    yt_ps, wo_sb[:, h, dt * 128:(dt + 1) * 128],
                    ot_tiles[c][:, h, :],
                    start=(h == 0), stop=(h == HPC - 1),
                    skip_group_check=True)

        def outproj_units(c, tail=False):
            """16 dt-group closures. The tail (chunk 3, after all other PE
            work) pipelines yt across four otherwise-idle PSUM rings, opens
            the first four groups split-phase (h0-h2 first) so the last
            head's normalization latency is hidden, and stores in 4-dt
            pieces to shorten the final drain."""
            if not tail:
                groups = []
                for dt in range(NT):
                    def grp(dt=dt, c=c):
                        yt_ps = ps_yt.tile([128, CH], F32, tag="yt")
                        op_mms(c, dt, yt_ps, range(HPC))
                        op_store(c, dt, yt_ps, False)
                    groups.append(grp)
                return groups

            # the ot ring is still held by head 3 until its normalization
            # completes, so it enters the rotation only from dt 5
            ring_seq = [(ps_yt, "yt"), (ps_st, "st"), (ps_yt, "yt"),
                        (ps_st, "st"), (ps_q, "q")] + [
                ((ps_ot, "ot"), (ps_yt, "yt"), (ps_st, "st"),
                 (ps_q, "q"))[(dt - 5) % 4] for dt in range(5, NT)]
            tiles = {}

            def tile_for(dt):
                pool, tg = ring_seq[dt]
                return pool.tile([128, CH], F32, tag=tg, name=f"ytt{dt}")

            def head_start():
                # groups 0-4: h0-h2 partials while head 3's norm finishes
                for dt in range(5):
                    tiles[dt] = tile_for(dt)
                    op_mms(c, dt, tiles[dt], range(HPC - 1))
                for dt in range(5):
                    op_mms(c, dt, tiles[dt], [HPC - 1])
                    op_store(c, dt, tiles[dt], True)

            groups = [head_start]
            for dt in range(5, NT):
                def grp(dt=dt, c=c):
                    yt_ps = tile_for(dt)
                    op_mms(c, dt, yt_ps, range(HPC))
                    op_store(c, dt, yt_ps, True)
                groups.append(grp)
            return groups

        op_queue = deque()   # output-projection filler groups (prev chunk)
        rdT_pending = [None]  # deferred normalization (PE transpose first)
        pend = [None]         # deferred last (j, t_, p, head ctx) flush

        def pop_rdT():
            if rdT_pending[0] is not None:
                rdT_pending[0]()
                rdT_pending[0] = None

        def flush_prev():
            """Emit mask + d-matmuls + AV for the deferred iteration; when it
            is a head's final iteration, chain the reciprocal and queue the
            rest of that head's normalization."""
            if pend[0] is None:
                return
            j, t_, p, pc, pnjt, d_ps, ot_ps, fin = pend[0]
            pend[0] = None
            o = 128 * t_ if t_ > 0 else 0
            if t_ >= 0:
                nc.vector.tensor_mul(
                    p[:, o:o + 128], p[:, o:o + 128],
                    masks_sb[:, t_, o:o + 128])
            # start=True marks the WHOLE 2KB PSUM bank pending-zero, so
            # only the bank's first group may carry it; sibling columns
            # zero-init via the pending-zero bytes it marked.
            for s_ in range(max(t_, 0), 4):
                nc.tensor.matmul(
                    d_ps[:, s_:s_ + 1],
                    p[:, s_ * 128:(s_ + 1) * 128], ones_sb,
                    start=(j == 0 and s_ == 0), stop=(j == 4 * pc + s_),
                    skip_group_check=True)
            nc.tensor.matmul(
                ot_ps[:, o:],
                v_sb[:, j * 128:(j + 1) * 128], p[:, o:],
                start=(j == 0), stop=(j == pnjt - 1),
                skip_group_check=True)
            if fin is not None:
                fin()

        def mark(label):
            _MARKS.append((label, nc.next_id()))

        for c in range(NCHUNK):
            col = c * CH
            njt = 4 * c + 4
            mark(f"fused{c}")

            # ---------- interleaved K / Q0 / V projection pass ----------
            # V is computed pre-transposed ([s, dv]) slice-major so each
            # 128-row slice finishes (and is copied out) early.
            k_acc = ps_ot.tile([128, CH], F32, tag="ot", name=f"k{c}")
            q_acc = ps_q.tile([128, CH], F32, tag="q", name=f"q{c}_0")
            sw_k = ps_q.tile([128, CH], F32, tag="q", name=f"swk{c}")
            sw_q = ps_q.tile([128, CH], F32, tag="q", name=f"swq{c}_0")
            v_ps = ps_st.tile([128, CH], F32, tag="st", name=f"v{c}")
            k_rope = rope_closures(k_acc, c, kt_sb[:, col:col + CH], sw_k)
            q_rope = rope_closures(q_acc, c, qt_tiles[c][:, 0, :], sw_q)
            q_mms = q_mm_closures(c, 0, q_acc)

            def v_mm(s_, t):
                # start only on the bank's first group (see the d-matmul
                # note on PSUM zero-region granularity)
                nc.tensor.matmul(
                    v_ps[:, s_ * 128:(s_ + 1) * 128],
                    x_tiles[c][:, t, s_ * 128:(s_ + 1) * 128],
                    wkv_sb[:, t, DH:2 * DH],
                    start=(t == 0 and s_ == 0), stop=(t == NT - 1),
                    skip_group_check=True)

            # phase 1: K, Q0 and V-slice 0, per x tile
            for t in range(NT):
                nc.tensor.matmul(
                    k_acc, wkv_sb[:, t, 0:DH], x_tiles[c][:, t, :],
                    start=(t == 0), stop=(t == NT - 1), skip_group_check=True)
                q_mms[t]()
                v_mm(0, t)
                if t == 2:
                    flush_prev()    # last AV + d of chunk c-1, head 3
            k_rope[0]()
            q_rope[0]()
            # phases 2-4: remaining V slices with rope work interleaved.
            # v_ps is copied out once at the end: PSUM dependencies are
            # bank-granular, so a per-slice copy would stall the next
            # slice's matmuls behind it.
            for t in range(NT):
                v_mm(1, t)
                if t == 1:
                    pop_rdT()       # normalization of chunk c-1, head 3
                elif t == 5:
                    k_rope[1]()     # sw_k matmul
                elif t == 9:
                    k_rope[2]()     # ta/tb mults
                elif t == 13:
                    k_rope[3]()     # kt add
            for t in range(NT):
                v_mm(2, t)
                if t == 5:
                    q_rope[1]()     # sw_q0
                elif t == 9:
                    q_rope[2]()
                elif t == 13:
                    q_rope[3]()
            for t in range(NT):
                v_mm(3, t)
            nc.scalar.copy(v_sb[:, col:col + CH], v_ps)

            # chunk 0's attention loops are too short to hide a full Q pass
            # + rope chain, so pre-emit head 1's here (the startup is
            # DMA-bound anyway); later heads then prefetch head h+2.
            q1_pre = c == 0
            if q1_pre:
                nq_acc = ps_q.tile([128, CH], F32, tag="q", name="q0_1")
                for u in q_mm_closures(0, 1, nq_acc):
                    u()
                nsw = ps_q.tile([128, CH], F32, tag="q", name="swq0_1")
                for u in rope_closures(nq_acc, 0, qt_tiles[0][:, 1, :], nsw,
                                       raw_on_act=False):
                    u()

            # ---------------- attention ----------------
            for h in range(HPC):
                mark(f"attn{c}_{h}")
                # filler units for the PE: the next not-yet-emitted head's
                # Q pass + rope
                fillers = deque()
                nh = h + 2 if q1_pre else h + 1
                if nh < HPC:
                    nq_acc = ps_q.tile([128, CH], F32, tag="q",
                                       name=f"q{c}_{nh}")
                    fillers.extend(q_mm_closures(c, nh, nq_acc))
                    nsw = ps_q.tile([128, CH], F32, tag="q",
                                    name=f"swq{c}_{nh}")
                    fillers.extend(
                        rope_closures(nq_acc, c, qt_tiles[c][:, nh, :], nsw,
                                      raw_on_act=False))

                d_full = ps_d.tile([128, CH], F32, tag="d", name=f"d{c}_{h}")
                d_ps = d_full[:, 0:4]
                ot_ps = ps_ot.tile([128, CH], F32, tag="ot", name=f"otp{c}_{h}")

                # closures for the head's normalization, chained off the
                # deferred final flush. bc[:, s*128+k][p] = rd[k, s] comes
                # from four tiny PE matmuls whose stationary operand is the
                # rd column free-broadcast against a bf16 identity -- the
                # compiler rejects narrow transposes and partition-offset
                # broadcasts, and this is cheaper anyway.
                rd = misc.tile([128, 4], F32, tag="rd")
                rdb = misc.tile([128, 4], BF16, tag="rdb")
                bc_full = ps_d.tile([128, CH], F32, tag="d", name=f"bc{c}_{h}")

                otf = misc.tile([128, CH], F32, tag="otf")

                def norm_deferred(rdb=rdb, bc=bc_full, ot_ps=ot_ps, otf=otf,
                                  c=c, h=h):
                    # stage O^T out of PSUM (a tensor op may read only
                    # one PSUM operand, and GPSIMD cannot access PSUM) --
                    # this also frees the ot bank early
                    nc.vector.tensor_copy(otf, ot_ps)
                    for s_ in range(4):
                        nc.tensor.matmul(
                            bc[:, s_ * 128:(s_ + 1) * 128],
                            rdb[:, s_:s_ + 1].broadcast_to([128, 128]),
                            ident_sb, start=(s_ == 0), stop=True,
                            skip_group_check=True)
                    nc.vector.tensor_mul(ot_tiles[c][:, h, :], otf, bc)

                def fin(rd=rd, rdb=rdb, d_ps=d_ps, norm_deferred=norm_deferred):
                    nc.vector.reciprocal(rd, d_ps)
                    nc.vector.tensor_copy(rdb, rd)
                    rdT_pending[0] = norm_deferred

                n_f = len(fillers)
                per_j = (n_f + min(4, max(1, njt - 1)) - 1) \
                    // min(4, max(1, njt - 1))

                for j in range(njt):
                    t_ = j - 4 * c
                    o = 128 * t_ if t_ > 0 else 0
                    if j == 1:
                        pop_rdT()   # normalization of the previous head
                    st_ps = ps_st.tile([128, CH], F32, tag="st")
                    nc.tensor.matmul(
                        st_ps[:, o:], kt_sb[:, j * 128:(j + 1) * 128],
                        qt_tiles[c][:, h, o:],
                        start=True, stop=True, skip_group_check=True)
                    p = ppool.tile([128, CH], BF16, tag="p")
                    nc.scalar.activation(p[:, o:], st_ps[:, o:], EXP, scale=SCALE)
                    for _ in range(per_j):
                        if fillers:
                            fillers.popleft()()
                    if op_queue and j >= 1 and (j % 2 == 1 or not fillers):
                        op_queue.popleft()()
                    flush_prev()
                    pend[0] = (j, t_, p, c, njt, d_ps, ot_ps,
                               fin if j == njt - 1 else None)
                while fillers:
                    fillers.popleft()()

            # queue this chunk's output projection for the next chunk
            # (chunk 3's runs at the drain and uses the tail variant)
            op_queue.extend(outproj_units(c, tail=(c == NCHUNK - 1)))

        # drain: chunk 3's last flush + normalization, then its output
        # projection
        mark("drain")
        flush_prev()
        mark("drain_norm")
        pop_rdT()
        mark("drain_op")
        i_op = 0
        while op_queue:
            mark(f"dop{i_op}")
            op_queue.popleft()()
            i_op += 1

    nc.compile()
    return nc


def _host_prep(x, freqs_cos, freqs_sin, wq, wk, wv, wo):
    """Build the 8 per-core input maps (bf16 operands)."""
    import ml_dtypes
    bf16 = ml_dtypes.bfloat16

    cos_t = np.ascontiguousarray(freqs_cos.T)  # [64, S]
    sin_t = np.ascontiguousarray(freqs_sin.T)
    cs = np.repeat(cos_t, 2, axis=0).astype(np.float32)        # [128, S]
    sn = np.repeat(sin_t, 2, axis=0).astype(np.float32)
    sn[0::2] *= -1.0
    cssn = np.concatenate([cs, sn], axis=1).astype(bf16)       # [128, 2S]

    pswap = np.zeros((DH, DH), dtype=np.float32)
    idx = np.arange(0, DH, 2)
    pswap[idx, idx + 1] = 1.0
    pswap[idx + 1, idx] = 1.0
    pswap = pswap.astype(bf16)

    ident = np.eye(128, dtype=np.float32).astype(bf16)

    # masks[t][jj, ii] = 1 where query ii >= key (128*t + jj), for the 4
    # diagonal key tiles of each 512-wide query chunk.
    ii = np.arange(CH)[None, :]
    jj = np.arange(128)[:, None]
    masks = np.concatenate(
        [(ii >= 128 * t + jj) for t in range(4)], axis=1
    ).astype(bf16)  # [128, 4*CH]

    xTs = [np.ascontiguousarray(x[b].T).astype(bf16) for b in range(2)]
    wq16 = wq.astype(bf16)
    wo16 = wo.astype(bf16)

    in_maps = []
    for core in range(N_CORES):
        b, g = divmod(core, HPC)
        wkv = np.concatenate(
            [wk[:, g * DH:(g + 1) * DH], wv[:, g * DH:(g + 1) * DH]], axis=1
        ).astype(bf16)
        in_maps.append({
            "xT": xTs[b],
            "wq": np.ascontiguousarray(wq16[:, g * HPC * DH:(g + 1) * HPC * DH]),
            "wkv": wkv,
            "wo": np.ascontiguousarray(wo16[g * HPC * DH:(g + 1) * HPC * DH, :]),
            "cssn": cssn, "pswap": pswap, "ident": ident, "masks": masks,
            "ones": np.ones((128, 1), dtype=bf16),
        })
    return in_maps


def kernel(x, freqs_cos, freqs_sin, mask, wq, wk, wv, wo):
    x = np.asarray(x, dtype=np.float32)
    freqs_cos = np.asarray(freqs_cos, dtype=np.float32)
    freqs_sin = np.asarray(freqs_sin, dtype=np.float32)
    wq = np.asarray(wq, dtype=np.float32)
    wk = np.asarray(wk, dtype=np.float32)
    wv = np.asarray(wv, dtype=np.float32)
    wo = np.asarray(wo, dtype=np.float32)

    if "nc" not in _CACHE:
        _CACHE["nc"] = _build()
    nc = _CACHE["nc"]

    in_maps = _host_prep(x, freqs_cos, freqs_sin, wq, wk, wv, wo)
    res = bass_utils.run_bass_kernel_spmd(nc, in_maps, core_ids=list(range(N_CORES)))

    out = np.empty((2, S, DM), dtype=np.float32)
    for b in range(2):
        acc = res.results[b * HPC]["yT"].astype(np.float32)
        for g in range(1, HPC):
            acc = acc + res.results[b * HPC + g]["yT"].astype(np.float32)
        out[b] = acc.T
    return out
